# revision 22
# baseline (speedup 1.0000x reference)
"""Trainium2 Bass kernel for AdaptiveSpectralFeatureRefinementCosine.

Math (per batch, pixel x, 3x3 window taps k, C=128 channels):
    nf(x) = max(||fused(:,x)||, 1e-12), ne(x) = max(||fe(:,x)||, 1e-12)
    cos(k,x) = <fe(:,x)/ne(x), fused(:,x+dk)/nf(x+dk)>
    w(k,x) = softmax_k cos(k,x)            (cos in [-1,1]: no max-subtract)
    out(c,x) = sum_k w(k,x)*fused(c,x+dk) + fe(c,x)

Sharding: B*H = 512 image rows -> 64 rows per core on 8 cores
(core = 2*b + rowhalf). Device gets fe slab (C,64,128) and zero-padded
fused slab (C,66,130) incl. halo -> no edge handling on device.

Device layout: C=128 on partitions, pixels on free dim. Blocks of 8 rows
(NF=1024 px) split into two 4-row halves for N<=512 matmuls; the halves'
matmuls use PE tile groups 0 and 32 (tile_position) so they overlap on
the systolic array. Softmax reciprocals use the fast approx DVE op; the
1/norm rows are collapsed to per-quadrant flat vectors by DMA so the
per-pixel broadcasts are plain ones-matmuls (no tiny partition-collapse
DMAs per group). fpn is kept twice (dj=0/2 aligned view + dj=1 shifted
copy) so every product hits the DVE 2x bf16 mode.
"""
import os
import sys

sys.path.insert(0, "/opt/trn_rl_repo")
import numpy as np

B, C, H, W = 4, 128, 128, 128
ROWS = 64                   # output rows per core
FR, FC = ROWS + 2, W + 2    # fused slab (66, 130)
NBLK, BR = 8, 8             # 8 blocks x 8 rows
NF = BR * W                 # 1024
HNF = NF // 2               # 512 (half-block, 4 rows)
HB = BR // 2                # 4

# fp normalize: 3-row groups, quarters aligned to groups (rows/quarter)
FP_QROWS = (18, 18, 15, 15)
FE_QROWS = (16, 16, 16, 16)

_CACHE = {}


def _build_nc(reps=1):
    from concourse import bass, tile, bacc

    mybir = bass.mybir
    F32 = mybir.dt.float32
    BF16 = mybir.dt.bfloat16
    MUL = mybir.AluOpType.mult
    ADD = mybir.AluOpType.add
    AF = mybir.ActivationFunctionType

    nc = bacc.Bacc(None, target_bir_lowering=False)
    fe_ext = nc.declare_dram_parameter("fe", [C, ROWS, W], F32, isOutput=False)
    fp_ext = nc.declare_dram_parameter("fp", [C, FR, FC], F32, isOutput=False)
    out_ext = nc.declare_dram_parameter("out", [C, ROWS, W], F32, isOutput=True)

    TAPS = [(di, dj) for di in range(3) for dj in range(3)]
    # engine split knobs: taps/ops listed run on gpsimd (pool), rest on DVE
    PR_POOL = set(int(c) for c in os.environ.get("PRP", ""))
    GK_POOL = set(int(c) for c in os.environ.get("GKP", "147"))
    AD_POOL = set(int(c) for c in os.environ.get("ADP", "3"))
    AN_POOL = os.environ.get("ANP", "0") == "1"
    OT_POOL = os.environ.get("OTP", "0") == "1"

    with tile.TileContext(nc) as tc:
        with (
            tc.tile_pool(name="big", bufs=1) as big,
            tc.tile_pool(name="cst", bufs=1) as cst,
            tc.tile_pool(name="wk", bufs=2) as wk,
            tc.tile_pool(name="rcpp", bufs=2) as rcpp,
            tc.tile_pool(name="gkp", bufs=2) as gkp,
            tc.tile_pool(name="psS", bufs=2, space="PSUM") as psS,
            tc.tile_pool(name="psV", bufs=2, space="PSUM") as psV,
            tc.tile_pool(name="psU", bufs=1, space="PSUM") as psU,
        ):
            fe_sb = big.tile([C, ROWS, W], F32)
            fp_sb = big.tile([C, FR, FC], F32)
            fen = big.tile([C, ROWS, W], BF16)
            fp_bf = big.tile([C, FR, FC], BF16)
            fpn0 = big.tile([C, FR, FC], BF16)     # normalized, dj=0/2 views
            fpn1 = big.tile([C, FR, W], BF16)      # shifted copy dj=1
            # flat copies spread over partition quadrants 0/32/64/96
            rf_fl = big.tile([97, FP_QROWS[0] * FC], BF16)
            re_fl = big.tile([97, FE_QROWS[0] * W], BF16)

            ones_row_f = cst.tile([97, C], F32)    # rows 0/32/64/96 = 1.0
            ones_row_b = cst.tile([97, C], BF16)
            ones41 = cst.tile([41, 1], BF16)       # sums lhsT (rows 0-8, 32-40)
            band9 = cst.tile([C, 17], BF16)        # sliding one-hot (9-col)
            band66 = cst.tile([C, 2 * FR - 1], BF16)
            band64 = cst.tile([C, 2 * ROWS - 1], BF16)
            e9t = cst.tile([41, 9 * C], BF16)      # row one-hots at base 0 / 32

            for ch in range(8):
                a, b2 = 8 * ch, 8 * (ch + 1)
                nc.sync.dma_start(fe_sb[:, a:b2, :], fe_ext[:, a:b2, :])
            for ch in range(6):
                a = 11 * ch
                b2 = min(FR, 11 * (ch + 1))
                nc.sync.dma_start(fp_sb[:, a:b2, :], fp_ext[:, a:b2, :])
            nc.vector.memset(ones_row_f[:], 0.0)
            nc.vector.memset(ones_row_b[:], 0.0)
            for q in range(4):
                nc.vector.memset(ones_row_f[32 * q:32 * q + 1, :], 1.0)
                nc.vector.memset(ones_row_b[32 * q:32 * q + 1, :], 1.0)
            nc.vector.memset(ones41[:], 0.0)
            nc.vector.memset(ones41[0:9, :], 1.0)
            nc.vector.memset(ones41[32:41, :], 1.0)
            nc.vector.memset(band9[:], 0.0)
            nc.vector.memset(band9[:, 8:9], 1.0)
            nc.vector.memset(band66[:], 0.0)
            nc.vector.memset(band66[:, FR - 1:FR], 1.0)
            nc.vector.memset(band64[:], 0.0)
            nc.vector.memset(band64[:, ROWS - 1:ROWS], 1.0)
            nc.vector.memset(e9t[:], 0.0)
            ones1 = cst.tile([1, C], BF16)
            nc.vector.memset(ones1[:], 1.0)
            eps_t = cst.tile([C, 1], F32)
            nc.vector.memset(eps_t[:], 1e-24)
            for k in range(9):
                nc.sync.dma_start(e9t[k:k + 1, C * k:C * (k + 1)], ones1[:])
                nc.sync.dma_start(e9t[32 + k:33 + k, C * k:C * (k + 1)],
                                  ones1[:])
            sums_ps = psU.tile([33, HNF], F32, tag="sums")
            nc.vector.memset(sums_ps[:], 1.0)

            # ---------------- compute (repeated for timing) ----------------
            from contextlib import nullcontext
            with (tc.For_i(0, reps, 1) if reps > 1 else nullcontext()):
              # raw bf16 cast of fp (halves on DVE / Act)
              nc.vector.tensor_copy(fp_bf[:, 0:33, :], fp_sb[:, 0:33, :])
              nc.scalar.copy(fp_bf[:, 33:FR, :], fp_sb[:, 33:FR, :])
              # -------- phase 0: norms, pipelined per quadrant --------
              def _norm_quadrant(src_sb, band, fl, q, r0, nr, width,
                                 dma_eng, sq_pool=False):
                  # squares for rows [r0, r0+nr) in <=6-row chunks
                  sqs = []
                  for c0 in range(0, nr, 6):
                      cn = min(6, nr - c0)
                      sq = wk.tile([C, cn, width], BF16, tag="sqf")
                      if sq_pool:
                          nc.gpsimd.tensor_tensor(
                              sq[:], src_sb[:, r0 + c0:r0 + c0 + cn, :],
                              src_sb[:, r0 + c0:r0 + c0 + cn, :], MUL)
                      else:
                          nc.scalar.activation(
                              sq[:], src_sb[:, r0 + c0:r0 + c0 + cn, :],
                              AF.Square)
                      sqs.append((sq, cn))
                  n2 = psS.tile([nr, width], F32, tag="s")
                  ly = 0
                  L = band.shape[1] // 2  # one-hot col position
                  for sq, cn in sqs:
                      for r in range(cn):
                          nc.tensor.matmul(
                              n2[:], band[:, L - ly:L - ly + nr], sq[:, r, :],
                              start=(ly == 0), stop=(ly == nr - 1))
                          ly += 1
                  nm = wk.tile([nr, width], F32, tag="nf2m")
                  nc.scalar.activation(nm[:], n2[:], AF.Sqrt,
                                       bias=eps_t[0:nr, :])
                  rt = wk.tile([nr, width], F32, tag="rtq")
                  nc.vector.reciprocal_approx_fast(rt[:], nm[:])
                  rtb = wk.tile([nr, width], BF16, tag="rtbq")
                  nc.scalar.copy(rtb[:], rt[:])
                  dma_eng.dma_start(fl[32 * q:32 * q + 1, 0:nr * width],
                                    rtb[:])

              r0f = r0e = 0
              for q in range(4):
                  nrf, nre = FP_QROWS[q], FE_QROWS[q]
                  _norm_quadrant(fp_sb, band66, rf_fl, q, r0f, nrf, FC,
                                 nc.sync if q % 2 == 0 else nc.scalar)
                  _norm_quadrant(fe_sb, band64, re_fl, q, r0e, nre, W,
                                 nc.scalar if q % 2 == 0 else nc.sync)
                  r0f += nrf
                  r0e += nre

              # -------- phase 0b: normalize + bf16 cast --------
              for g in range(16):                       # fe: 4-row groups
                  q, lg = g // 4, g % 4
                  bc = psV.tile([C, 4, W], F32, tag="vb")
                  nc.tensor.matmul(
                      bc[:].rearrange("c r x -> c (r x)"),
                      ones_row_b[32 * q:32 * q + 1, :],
                      re_fl[32 * q:32 * q + 1, HNF * lg:HNF * (lg + 1)],
                      tile_position=(32 * q, 0))
                  nc.vector.tensor_tensor(
                      fen[:, 4 * g:4 * (g + 1), :],
                      fe_sb[:, 4 * g:4 * (g + 1), :], bc[:], MUL)
              gpq = (0, 6, 12, 17)                      # fp group start/quarter
              for g in range(22):                       # fp: 3-row groups
                  q = sum(1 for s in gpq[1:] if g >= s)
                  lg = g - gpq[q]
                  bc = psV.tile([C, 3, FC], F32, tag="vb")
                  nc.tensor.matmul(
                      bc[:].rearrange("c r x -> c (r x)"),
                      ones_row_b[32 * q:32 * q + 1, :],
                      rf_fl[32 * q:32 * q + 1, 3 * FC * lg:3 * FC * (lg + 1)],
                      tile_position=(32 * q, 0))
                  nc.vector.tensor_tensor(
                      fpn0[:, 3 * g:3 * (g + 1), :],
                      fp_sb[:, 3 * g:3 * (g + 1), :], bc[:], MUL)
              # aligned dj=1 shifted copy (chunked: unblocks early rows)
              r0 = 0
              for nr in FP_QROWS:
                  nc.vector.tensor_copy(fpn1[:, r0:r0 + nr, :],
                                        fpn0[:, r0:r0 + nr, 1:1 + W])
                  r0 += nr

              # ---------------- main loop ----------------
              for ib in range(NBLK):
                  i0 = BR * ib
                  s_ps = psS.tile([41, HNF], F32, tag="s")
                  for k, (di, dj) in enumerate(TAPS):
                      pr = wk.tile([C, BR, W], BF16, tag="pr")
                      src = (fpn1[:, i0 + di:i0 + di + BR, :] if dj == 1 else
                             fpn0[:, i0 + di:i0 + di + BR, dj:dj + W])
                      peng = nc.gpsimd if k in PR_POOL else nc.vector
                      peng.tensor_tensor(pr[:], fen[:, i0:i0 + BR, :], src,
                                         MUL)
                      for h in range(2):
                          nc.tensor.matmul(
                              s_ps[32 * h:32 * h + 9, :],
                              band9[:, 8 - k:17 - k],
                              pr[:, HB * h:HB * (h + 1), :],
                              start=(k == 0), stop=(k == 8),
                              skip_group_check=True,
                              tile_position=(0, 32 * h))

                  wexp = wk.tile([41, NF], BF16, tag="wexp")
                  nc.scalar.activation(wexp[0:9, 0:HNF], s_ps[0:9, :], AF.Exp)
                  nc.scalar.activation(wexp[32:41, HNF:NF], s_ps[32:41, :],
                                       AF.Exp)
                  nc.tensor.matmul(sums_ps[0:1, :], ones41[0:9, :],
                                   wexp[0:9, 0:HNF], tile_position=(0, 0))
                  nc.tensor.matmul(sums_ps[32:33, :], ones41[32:41, :],
                                   wexp[32:41, HNF:NF],
                                   tile_position=(32, 32))
                  rcp = rcpp.tile([33, HNF], F32, tag="rcp")
                  nc.vector.reciprocal_approx_fast(rcp[:], sums_ps[:])
                  rb = psV.tile([C, BR, W], F32, tag="vb")
                  for h in range(2):
                      nc.tensor.matmul(
                          rb[:, HB * h:HB * (h + 1), :].rearrange(
                              "c r x -> c (r x)"),
                          ones_row_f[32 * h:32 * h + 1, :],
                          rcp[32 * h:32 * h + 1, :],
                          tile_position=(32 * h, 0))
                  rb_sb = gkp.tile([C, BR, W], BF16, tag="rbs")
                  nc.scalar.copy(rb_sb[:], rb[:])

                  # aggregation: per-tap broadcast + mult, two add-chains
                  # interleaved with production to keep slot reuse unblocked
                  gs = []
                  chains = [None, None]
                  ei = 0

                  def _emit_gk(k, di, dj):
                      vb = psV.tile([C, BR, W], F32, tag="vb")
                      for h in range(2):
                          nc.tensor.matmul(
                              vb[:, HB * h:HB * (h + 1), :].rearrange(
                                  "c r x -> c (r x)"),
                              e9t[32 * h:32 * h + 9, C * k:C * (k + 1)],
                              wexp[32 * h:32 * h + 9,
                                   HNF * h:HNF * (h + 1)],
                              tile_position=(32 * h, 0))
                      vbs = gkp.tile([C, BR, W], BF16, tag=f"vbs{k % 2}")
                      nc.scalar.copy(vbs[:], vb[:])
                      gk = gkp.tile([C, BR, W], BF16, tag=f"gk{k % 3}")
                      meng = nc.gpsimd if k in GK_POOL else nc.vector
                      meng.tensor_tensor(
                          gk[:], fp_bf[:, i0 + di:i0 + di + BR, dj:dj + W],
                          vbs[:], MUL)
                      gs.append(gk)

                  def _emit_add(c):
                      nonlocal ei
                      t = gkp.tile([C, BR, W], BF16, tag=f"ch{c}")
                      eng = nc.gpsimd if ei in AD_POOL else nc.vector
                      prev = chains[c] if chains[c] is not None else gs[c]
                      eng.tensor_tensor(t[:], prev[:], gs[2 + ei][:], ADD)
                      chains[c] = t
                      ei += 1

                  for k, (di, dj) in enumerate(TAPS):
                      _emit_gk(k, di, dj)
                      # after gk j>=2 exists, fold it into chain (j-2)%2... keep
                      # order: g0,g1,g2 -> a=g0+g2; g3 -> b=g1+g3; g4 -> a+=g4..
                      if k >= 2:
                          _emit_add(k % 2)
                  acc = gkp.tile([C, BR, W], BF16, tag="ch0")
                  eng = nc.gpsimd if 7 in AD_POOL else nc.vector
                  eng.tensor_tensor(acc[:], chains[0][:], chains[1][:], ADD)

                  an = gkp.tile([C, BR, W], BF16, tag="gk0")
                  aeng = nc.gpsimd if AN_POOL else nc.vector
                  aeng.tensor_tensor(an[:], acc[:], rb_sb[:], MUL)
                  for h in range(2):
                      ot = gkp.tile([C, HB, W], F32, tag="ot")
                      eng = (nc.gpsimd if OT_POOL and (2 * ib + h) % 2
                             else nc.vector)
                      eng.tensor_tensor(
                          ot[:], an[:, HB * h:HB * (h + 1), :],
                          fe_sb[:, i0 + HB * h:i0 + HB * (h + 1), :], ADD)
                      nc.sync.dma_start(
                          out_ext[:, i0 + HB * h:i0 + HB * (h + 1), :], ot[:])
    nc.finalize()
    return nc


def _get_nc(reps=1):
    key = f"nc{reps}"
    if key not in _CACHE:
        _CACHE[key] = _build_nc(reps)
    return _CACHE[key]


def _shard_inputs(fe_lv, fused_features):
    fe_lv = np.ascontiguousarray(fe_lv, dtype=np.float32)
    fp = np.zeros((B, C, H + 2, W + 2), dtype=np.float32)
    fp[:, :, 1:-1, 1:-1] = fused_features
    in_maps = []
    for core in range(8):
        b, half = core // 2, core % 2
        r0 = half * ROWS
        in_maps.append({
            "fe": np.ascontiguousarray(fe_lv[b, :, r0:r0 + ROWS, :]),
            "fp": np.ascontiguousarray(fp[b, :, r0:r0 + FR, :]),
        })
    return in_maps


def kernel(fe_lv, fused_features):
    from concourse.bass_utils import run_bass_kernel_spmd

    nc = _get_nc()
    in_maps = _shard_inputs(fe_lv, fused_features)
    res = run_bass_kernel_spmd(nc, in_maps, core_ids=list(range(8)))
    out = np.empty((B, C, H, W), dtype=np.float32)
    for core in range(8):
        b, half = core // 2, core % 2
        out[b, :, half * ROWS:half * ROWS + ROWS, :] = res.results[core]["out"]
    return out
